# revision 20
# baseline (speedup 1.0000x reference)
"""Trainium2 Bass kernel for BERTForContrastiveLearningForTokenMetric loss.

Math: the reference loss factors into masked per-token sums:
    proto = (sum_{ent} x_t) / n_ent
    loss  = (sum_{nz} x_t/||x_t||) . proto / (||proto|| * n_tok)
so one pass over the contributing tokens suffices.  Host-side prep:
  - tokens with label == 0 (and label != entity_id) contribute to
    neither sum and are dropped (~10% of tokens),
  - survivors are cast to bf16 (the on-chip matmul dtype anyway, so
    this costs no accuracy vs casting in the DMA) and padded to a
    multiple of 8*128,
  - split evenly across the 8 cores; each core's tokens are stored as
    per-DMA-chunk contiguous blocks so HBM sees sequential reads.
Each core produces a [4, 768] partial (ent/nz sums split over two
PSUM bank sets); the host sums all partials and combines in fp64.

Per-core device pipeline (G groups of 128 tokens, token = (g, p)):
    HWDGE bf16 chunk DMAs (small head, ~6-group middle, tiny tail;
    FIFO on the sync ring -> in-order completion) stream x into SBUF,
    aux rides the idle gpsimd SWDGE ring, ACT activation tables are
    preloaded with dummy ops so they stay off the critical path.
    Per group DVE/ACT alternate fused square+accumulate -> sq;
    per 2-group chunklet: DVE reciprocal -> ACT sqrt -> DVE nz-mask
    scale (bf16 weights in place in the aux tile);
    PE bf16 matmuls accumulate into 4 PSUM banks (even groups ->
    bank set A, odd -> B; consecutive matmuls never share a bank);
    last group: split-D norm on DVE+ACT to shorten the tail;
    final: DVE bank-set adds (PSUM->SBUF), one HWDGE store.
"""

import numpy as np

B, S, D = 64, 512, 768
N_CORES = 8
P = 128                              # SBUF partitions / tokens per group
G_FULL = (B * S) // (N_CORES * P)    # 32 groups/core with no compaction

_CACHE = {}


def _chunk_sizes(G):
    """DMA chunk widths: small head (compute starts early), ~6-group
    middle (amortizes the ~0.7us per-issue cost on the sync ring), tiny
    tail (short post-stream tail)."""
    if G <= 9:
        return [1] * G
    head = [1, 1, 2, 3]
    tail = [2, 1, 1]
    mid_total = G - 11
    n_mid = -(-mid_total // 6)
    mid = []
    rem = mid_total
    for i in range(n_mid):
        take = -(-rem // (n_mid - i))
        mid.append(take)
        rem -= take
    return head + mid + tail


def _chunk_bounds(G):
    bounds = []
    g0 = 0
    for w in _chunk_sizes(G):
        bounds.append((g0, g0 + w))
        g0 += w
    return bounds


def _tile_program(nc, x_chunks, aux_h, out_h, G):
    """Emit the per-core Tile program.

    x_chunks: list of DRAM handles [P, W_k, D] bf16, chunk k of the
              token shard (token t = g*128 + p, g = global group)
    aux_h [P, G, 2] bf16 : (ent_mask, nz_mask) per token
    out_h [2, 2D] f32    : partials, cols 0:D bank set A, D:2D set B
    """
    import concourse.tile as tile
    from concourse import mybir

    f32 = mybir.dt.float32
    bf16 = mybir.dt.bfloat16
    OP = mybir.AluOpType
    AF = mybir.ActivationFunctionType
    HALF = 384

    bounds = _chunk_bounds(G)

    with tile.TileContext(nc) as tc:
        with (
            tc.tile_pool(name="sb", bufs=1) as sb,
            tc.tile_pool(name="psum", bufs=1, space="PSUM") as psp,
        ):
            x_sb = sb.tile([P, G, D], bf16)
            aux_sb = sb.tile([P, G, 2], bf16)
            sq = sb.tile([P, G], f32)
            sq2 = sb.tile([P, 2], f32)
            isq = sb.tile([P, G], f32)
            inv = sb.tile([P, G], f32)
            dump_v = sb.tile([P, D], bf16)
            dump_a = sb.tile([P, D], bf16)
            warm = sb.tile([P, 1], f32)
            warm2 = sb.tile([P, 1], f32)
            out_sb = sb.tile([2, 2 * D], f32)
            pA512 = psp.tile([2, 512], f32)
            pB512 = psp.tile([2, 512], f32)
            pA256 = psp.tile([2, 256], f32)
            pB256 = psp.tile([2, 256], f32)

            # x chunks queued up-front on the sync HWDGE ring (FIFO ->
            # in-order completion); aux rides the idle gpsimd SWDGE ring.
            for (a, b), xc in zip(bounds, x_chunks):
                nc.sync.dma_start(out=x_sb[:, a:b, :], in_=xc[:])
            nc.gpsimd.dma_start(out=aux_sb[:], in_=aux_h[:])

            # preload both ACT function tables (Square, Sqrt) so the
            # lazy table loads stay off the weight-chain critical path
            nc.vector.memset(warm[:], 1.0)
            nc.scalar.activation(out=warm2[:], in_=warm[:], func=AF.Square)
            nc.scalar.activation(out=warm2[:], in_=warm[:], func=AF.Sqrt)

            # per-bank start/stop bookkeeping for the 4 PSUM tiles
            evens = list(range(0, G, 2))
            odds = list(range(1, G, 2))
            first_of = {}
            last_of = {}
            for g in evens:
                first_of[g] = "A" if g == evens[0] else None
                last_of[g] = "A" if g == evens[-1] else None
            for g in odds:
                first_of[g] = "B" if g == odds[0] else first_of.get(g)
                last_of[g] = "B" if g == odds[-1] else last_of.get(g)

            def mm(g):
                p5 = pA512 if g % 2 == 0 else pB512
                p2 = pA256 if g % 2 == 0 else pB256
                tag = "A" if g % 2 == 0 else "B"
                first = first_of.get(g) == tag
                last = last_of.get(g) == tag
                nc.tensor.matmul(
                    p5[:], aux_sb[:, g, :], x_sb[:, g, 0:512],
                    start=first, stop=last,
                )
                nc.tensor.matmul(
                    p2[:], aux_sb[:, g, :], x_sb[:, g, 512:768],
                    start=first, stop=last,
                )

            # weight-chain chunklets: small head (matmuls start early),
            # 4-group middle (fewer chain ops), 1-group tail.
            widths = [min(2, G)]
            rem = G - widths[0] - (1 if G > 2 else 0)
            while rem > 0:
                w = min(4, rem)
                widths.append(w)
                rem -= w
            if G > 2:
                widths.append(1)
            clets = []
            a0 = 0
            for w in widths:
                clets.append((a0, a0 + w))
                a0 += w
            sq_idx = 0
            for a, b in clets:
                for g in range(a, b):
                    if g == G - 1:
                        # final group: split D across DVE and ACT
                        nc.vector.scalar_tensor_tensor(
                            out=dump_v[:, 0:HALF],
                            in0=x_sb[:, g, 0:HALF],
                            scalar=1.0,
                            in1=x_sb[:, g, 0:HALF],
                            op0=OP.mult,
                            op1=OP.mult,
                            accum_out=sq2[:, 0:1],
                        )
                        nc.scalar.activation(
                            out=dump_a[:, 0 : D - HALF],
                            in_=x_sb[:, g, HALF:D],
                            func=AF.Square,
                            accum_out=sq2[:, 1:2],
                        )
                        nc.vector.tensor_tensor(
                            out=sq[:, g : g + 1],
                            in0=sq2[:, 0:1],
                            in1=sq2[:, 1:2],
                            op=OP.add,
                        )
                    elif sq_idx % 7 in (0, 2, 4, 5):
                        nc.vector.scalar_tensor_tensor(
                            out=dump_v[:],
                            in0=x_sb[:, g, :],
                            scalar=1.0,
                            in1=x_sb[:, g, :],
                            op0=OP.mult,
                            op1=OP.mult,
                            accum_out=sq[:, g : g + 1],
                        )
                    else:
                        nc.scalar.activation(
                            out=dump_a[:],
                            in_=x_sb[:, g, :],
                            func=AF.Square,
                            accum_out=sq[:, g : g + 1],
                        )
                    sq_idx += 1

                # 1/||x|| for the chunklet, then scale the nz mask in place
                nc.vector.reciprocal(out=isq[:, a:b], in_=sq[:, a:b])
                nc.scalar.activation(
                    out=inv[:, a:b], in_=isq[:, a:b], func=AF.Sqrt
                )
                nc.vector.tensor_tensor(
                    out=aux_sb[:, a:b, 1],
                    in0=aux_sb[:, a:b, 1],
                    in1=inv[:, a:b],
                    op=OP.mult,
                )
                for g in range(a, b):
                    mm(g)

            # drain all four PSUM tiles (DVE + ACT in parallel); the host
            # folds the A (cols 0:768) and B (cols 768:1536) halves
            nc.vector.tensor_copy(out=out_sb[:, 0:512], in_=pA512[:])
            nc.scalar.copy(out=out_sb[:, D : D + 512], in_=pB512[:])
            nc.vector.tensor_copy(out=out_sb[:, D + 512 : 2 * D], in_=pB256[:])
            nc.scalar.copy(out=out_sb[:, 512:768], in_=pA256[:])
            nc.sync.dma_start(out=out_h[:], in_=out_sb[:])


def _dram_tensors(nc, G):
    from concourse import mybir

    f32 = mybir.dt.float32
    bf16 = mybir.dt.bfloat16
    x_chunks = [
        nc.dram_tensor(f"x{k}", [P, b - a, D], bf16, kind="ExternalInput")
        for k, (a, b) in enumerate(_chunk_bounds(G))
    ]
    aux = nc.dram_tensor("aux", [P, G, 2], bf16, kind="ExternalInput")
    out = nc.dram_tensor("out", [2, 2 * D], f32, kind="ExternalOutput")
    return x_chunks, aux, out


def _build(G):
    """Manual module build, used for CoreSim validation and timing."""
    import concourse.bacc as bacc

    nc = bacc.Bacc("TRN2", target_bir_lowering=False, debug=False)
    x_chunks, aux, out = _dram_tensors(nc, G)
    _tile_program(nc, x_chunks, aux, out, G)
    nc.finalize()
    return nc


def _get_nc(G=None):
    if G is None:
        G = _CACHE.get("G", G_FULL)
    key = ("nc", G)
    if key not in _CACHE:
        _CACHE[key] = _build(G)
    return _CACHE[key]


def _get_sharded_fn(G):
    """bass_jit kernel shard_mapped over the 8 cores (the proven exec path)."""
    key = ("fn", G)
    if key in _CACHE:
        return _CACHE[key]
    import jax
    from jax.sharding import Mesh, PartitionSpec
    from concourse.bass2jax import bass_jit, bass_shard_map
    from concourse import mybir

    f32 = mybir.dt.float32
    n_chunks = len(_chunk_bounds(G))

    # bass_jit inspects the body's signature, so generate explicit args
    arg_names = [f"x{k}" for k in range(n_chunks)] + ["aux"]
    src = (
        f"def _body(nc, {', '.join(arg_names)}):\n"
        f"    out = nc.dram_tensor('out', [2, 2 * D], f32, kind='ExternalOutput')\n"
        f"    _tile_program(nc, [{', '.join(arg_names[:-1])}], aux, out, G)\n"
        f"    return out\n"
    )
    ns = {"D": D, "f32": f32, "_tile_program": _tile_program, "G": G}
    exec(src, ns)
    body = bass_jit(ns["_body"])

    devices = jax.devices()[:N_CORES]
    mesh = Mesh(np.asarray(devices), ("core",))
    fn = bass_shard_map(
        body,
        mesh=mesh,
        in_specs=tuple(PartitionSpec("core") for _ in range(n_chunks + 1)),
        out_specs=PartitionSpec("core"),
    )
    _CACHE[key] = fn
    return fn


def _make_in_maps(logits, labels, entity_id):
    from concourse import mybir

    BF16 = mybir.dt.np(mybir.dt.bfloat16)

    lg = np.asarray(logits).astype(np.float32, copy=False).reshape(B * S, D)
    labels = np.asarray(labels).reshape(B, S)
    eid = int(np.asarray(entity_id))

    pos_ok = np.arange(S)[None, :] != 0
    ent = ((labels == eid) & pos_ok).reshape(-1)
    nz = (labels != 0).reshape(-1)
    c1 = max(float(ent.sum()), 1.0)
    c2 = max(float(nz.sum()), 1.0)

    # drop tokens that contribute to neither sum
    keep = nz | ent
    idx = np.nonzero(keep)[0]
    K = idx.size
    G = max(-(-K // (N_CORES * P)), 1)
    cap = N_CORES * P * G

    xk = np.ones((cap, D), dtype=BF16)           # pad rows: nonzero norm
    entk = np.zeros(cap, dtype=BF16)
    nzk = np.zeros(cap, dtype=BF16)
    xk[:K] = lg[idx].astype(BF16)
    entk[:K] = ent[idx].astype(BF16)
    nzk[:K] = nz[idx].astype(BF16)

    bounds = _chunk_bounds(G)
    tok_per_core = P * G
    in_maps = []
    for c in range(N_CORES):
        sl = slice(c * tok_per_core, (c + 1) * tok_per_core)
        x = xk[sl].reshape(G, P, D).transpose(1, 0, 2)  # [P, G, D] view
        m = {
            f"x{k}": np.ascontiguousarray(x[:, a:b, :])
            for k, (a, b) in enumerate(bounds)
        }
        m["aux"] = np.ascontiguousarray(
            np.stack([entk[sl], nzk[sl]], axis=-1)
            .reshape(G, P, 2)
            .transpose(1, 0, 2)
        )  # [P, G, 2]
        in_maps.append(m)

    _CACHE["G"] = G
    return in_maps, c1, c2


def _combine(partials, c1, c2):
    """partials: list of [2, D] float arrays (one per core)."""
    acc = np.zeros((2, D), dtype=np.float64)
    for p in partials:
        p = np.asarray(p, dtype=np.float64)
        if p.shape[-1] == 2 * D:
            p = p[:, 0:D] + p[:, D : 2 * D]
        acc += p
    v1, v2 = acc[0], acc[1]
    proto = v1 / c1
    pn = float(np.sqrt((proto * proto).sum()))
    if pn < 1e-30:
        return np.float32(0.0)
    loss = float(v2 @ proto) / (pn * c2)
    return np.float32(loss)


def _run_hw(in_maps):
    """Run the 8-core shard_map; returns list of [2, D] partials."""
    G = _CACHE.get("G", G_FULL)
    fn = _get_sharded_fn(G)
    names = [f"x{k}" for k in range(len(_chunk_bounds(G)))] + ["aux"]
    args = [
        np.concatenate([m[name] for m in in_maps], axis=0) for name in names
    ]
    out = np.asarray(fn(*args))  # [2 * N_CORES, 2 * D]
    return [out[2 * c : 2 * c + 2] for c in range(N_CORES)]


def kernel(logits, labels, entity_id):
    in_maps, c1, c2 = _make_in_maps(logits, labels, entity_id)
    partials = _run_hw(in_maps)
    return _combine(partials, c1, c2)


# revision 21
# speedup vs baseline: 1.1278x; 1.1278x over previous
"""Trainium2 Bass kernel for BERTForContrastiveLearningForTokenMetric loss.

Math: the reference loss factors into masked per-token sums:
    proto = (sum_{ent} x_t) / n_ent
    loss  = (sum_{nz} x_t/||x_t||) . proto / (||proto|| * n_tok)
so one pass over the contributing tokens suffices.  Host-side prep:
  - tokens with label == 0 (and label != entity_id) contribute to
    neither sum and are dropped (~10% of tokens),
  - survivors are cast to bf16 (the on-chip matmul dtype anyway, so
    this costs no accuracy vs casting in the DMA) and padded to a
    multiple of 8*128,
  - split evenly across the 8 cores; each core's tokens are stored as
    per-DMA-chunk contiguous blocks so HBM sees sequential reads.
Each core produces a [4, 768] partial (ent/nz sums split over two
PSUM bank sets); the host sums all partials and combines in fp64.

Per-core device pipeline (G groups of 128 tokens, token = (g, p)):
    HWDGE bf16 chunk DMAs (small head, ~6-group middle, tiny tail;
    FIFO on the sync ring -> in-order completion) stream x into SBUF,
    aux rides the idle gpsimd SWDGE ring, ACT activation tables are
    preloaded with dummy ops so they stay off the critical path.
    Per group DVE/ACT alternate fused square+accumulate -> sq;
    per 2-group chunklet: DVE reciprocal -> ACT sqrt -> DVE nz-mask
    scale (bf16 weights in place in the aux tile);
    PE bf16 matmuls accumulate into 4 PSUM banks (even groups ->
    bank set A, odd -> B; consecutive matmuls never share a bank);
    last group: split-D norm on DVE+ACT to shorten the tail;
    final: DVE bank-set adds (PSUM->SBUF), one HWDGE store.
"""

import numpy as np

B, S, D = 64, 512, 768
N_CORES = 8
P = 128                              # SBUF partitions / tokens per group
G_FULL = (B * S) // (N_CORES * P)    # 32 groups/core with no compaction

_CACHE = {}


def _chunk_sizes(G):
    """DMA chunk widths: small head (compute starts early), ~6-group
    middle (amortizes the ~0.7us per-issue cost on the sync ring), tiny
    tail (short post-stream tail)."""
    if G <= 9:
        return [1] * G
    head = [2, 3]
    tail = [2, 1]
    mid_total = G - 8
    n_mid = -(-mid_total // 6)
    mid = []
    rem = mid_total
    for i in range(n_mid):
        take = -(-rem // (n_mid - i))
        mid.append(take)
        rem -= take
    return head + mid + tail


def _chunk_bounds(G):
    bounds = []
    g0 = 0
    for w in _chunk_sizes(G):
        bounds.append((g0, g0 + w))
        g0 += w
    return bounds


def _tile_program(nc, x_chunks, aux_h, out_h, G):
    """Emit the per-core Tile program.

    x_chunks: list of DRAM handles [P, W_k, D] bf16, chunk k of the
              token shard (token t = g*128 + p, g = global group)
    aux_h [P, G, 2] bf16 : (ent_mask, nz_mask) per token
    out_h [2, 2D] f32    : partials, cols 0:D bank set A, D:2D set B
    """
    import concourse.tile as tile
    from concourse import mybir

    f32 = mybir.dt.float32
    bf16 = mybir.dt.bfloat16
    OP = mybir.AluOpType
    AF = mybir.ActivationFunctionType
    HALF = 384

    bounds = _chunk_bounds(G)

    with tile.TileContext(nc) as tc:
        with (
            tc.tile_pool(name="sb", bufs=1) as sb,
            tc.tile_pool(name="psum", bufs=1, space="PSUM") as psp,
        ):
            x_sb = sb.tile([P, G, D], bf16)
            aux_sb = sb.tile([P, G, 2], bf16)
            sq = sb.tile([P, G], f32)
            sq2 = sb.tile([P, 2], f32)
            isq = sb.tile([P, G], f32)
            inv = sb.tile([P, G], f32)
            dump_v = sb.tile([P, D], bf16)
            dump_a = sb.tile([P, D], bf16)
            warm = sb.tile([P, 1], f32)
            warm2 = sb.tile([P, 1], f32)
            out_sb = sb.tile([2, 2 * D], f32)
            pA512 = psp.tile([2, 512], f32)
            pB512 = psp.tile([2, 512], f32)
            pA256 = psp.tile([2, 256], f32)
            pB256 = psp.tile([2, 256], f32)

            # x chunks queued up-front on the sync HWDGE ring (FIFO ->
            # in-order completion); aux rides the idle gpsimd SWDGE ring.
            for (a, b), xc in zip(bounds, x_chunks):
                nc.sync.dma_start(out=x_sb[:, a:b, :], in_=xc[:])
            nc.gpsimd.dma_start(out=aux_sb[:], in_=aux_h[:])

            # preload both ACT function tables (Square, Sqrt) so the
            # lazy table loads stay off the weight-chain critical path
            nc.vector.memset(warm[:], 1.0)
            nc.scalar.activation(out=warm2[:], in_=warm[:], func=AF.Square)
            nc.scalar.activation(out=warm2[:], in_=warm[:], func=AF.Sqrt)

            # per-bank start/stop bookkeeping for the 4 PSUM tiles
            evens = list(range(0, G, 2))
            odds = list(range(1, G, 2))
            first_of = {}
            last_of = {}
            for g in evens:
                first_of[g] = "A" if g == evens[0] else None
                last_of[g] = "A" if g == evens[-1] else None
            for g in odds:
                first_of[g] = "B" if g == odds[0] else first_of.get(g)
                last_of[g] = "B" if g == odds[-1] else last_of.get(g)

            def mm(g):
                p5 = pA512 if g % 2 == 0 else pB512
                p2 = pA256 if g % 2 == 0 else pB256
                tag = "A" if g % 2 == 0 else "B"
                first = first_of.get(g) == tag
                last = last_of.get(g) == tag
                nc.tensor.matmul(
                    p5[:], aux_sb[:, g, :], x_sb[:, g, 0:512],
                    start=first, stop=last,
                )
                nc.tensor.matmul(
                    p2[:], aux_sb[:, g, :], x_sb[:, g, 512:768],
                    start=first, stop=last,
                )

            # weight-chain chunklets: small head (matmuls start early),
            # 4-group middle (fewer chain ops), 1-group tail.
            widths = [min(2, G)]
            rem = G - widths[0] - (1 if G > 2 else 0)
            while rem > 0:
                w = min(2, rem)
                widths.append(w)
                rem -= w
            if G > 2:
                widths.append(1)
            clets = []
            a0 = 0
            for w in widths:
                clets.append((a0, a0 + w))
                a0 += w
            sq_idx = 0
            for a, b in clets:
                for g in range(a, b):
                    if g == G - 1:
                        # final group: split D across DVE and ACT
                        nc.vector.scalar_tensor_tensor(
                            out=dump_v[:, 0:HALF],
                            in0=x_sb[:, g, 0:HALF],
                            scalar=1.0,
                            in1=x_sb[:, g, 0:HALF],
                            op0=OP.mult,
                            op1=OP.mult,
                            accum_out=sq2[:, 0:1],
                        )
                        nc.scalar.activation(
                            out=dump_a[:, 0 : D - HALF],
                            in_=x_sb[:, g, HALF:D],
                            func=AF.Square,
                            accum_out=sq2[:, 1:2],
                        )
                        nc.vector.tensor_tensor(
                            out=sq[:, g : g + 1],
                            in0=sq2[:, 0:1],
                            in1=sq2[:, 1:2],
                            op=OP.add,
                        )
                    elif sq_idx % 7 in (0, 2, 4, 5):
                        nc.vector.scalar_tensor_tensor(
                            out=dump_v[:],
                            in0=x_sb[:, g, :],
                            scalar=1.0,
                            in1=x_sb[:, g, :],
                            op0=OP.mult,
                            op1=OP.mult,
                            accum_out=sq[:, g : g + 1],
                        )
                    else:
                        nc.scalar.activation(
                            out=dump_a[:],
                            in_=x_sb[:, g, :],
                            func=AF.Square,
                            accum_out=sq[:, g : g + 1],
                        )
                    sq_idx += 1

                # 1/||x|| for the chunklet, then scale the nz mask in place
                nc.vector.reciprocal(out=isq[:, a:b], in_=sq[:, a:b])
                nc.scalar.activation(
                    out=inv[:, a:b], in_=isq[:, a:b], func=AF.Sqrt
                )
                nc.vector.tensor_tensor(
                    out=aux_sb[:, a:b, 1],
                    in0=aux_sb[:, a:b, 1],
                    in1=inv[:, a:b],
                    op=OP.mult,
                )
                for g in range(a, b):
                    mm(g)

            # drain all four PSUM tiles (DVE + ACT in parallel); the host
            # folds the A (cols 0:768) and B (cols 768:1536) halves
            nc.vector.tensor_copy(out=out_sb[:, 0:512], in_=pA512[:])
            nc.scalar.copy(out=out_sb[:, D : D + 512], in_=pB512[:])
            nc.vector.tensor_copy(out=out_sb[:, D + 512 : 2 * D], in_=pB256[:])
            nc.scalar.copy(out=out_sb[:, 512:768], in_=pA256[:])
            nc.sync.dma_start(out=out_h[:], in_=out_sb[:])


def _dram_tensors(nc, G):
    from concourse import mybir

    f32 = mybir.dt.float32
    bf16 = mybir.dt.bfloat16
    x_chunks = [
        nc.dram_tensor(f"x{k}", [P, b - a, D], bf16, kind="ExternalInput")
        for k, (a, b) in enumerate(_chunk_bounds(G))
    ]
    aux = nc.dram_tensor("aux", [P, G, 2], bf16, kind="ExternalInput")
    out = nc.dram_tensor("out", [2, 2 * D], f32, kind="ExternalOutput")
    return x_chunks, aux, out


def _build(G):
    """Manual module build, used for CoreSim validation and timing."""
    import concourse.bacc as bacc

    nc = bacc.Bacc("TRN2", target_bir_lowering=False, debug=False)
    x_chunks, aux, out = _dram_tensors(nc, G)
    _tile_program(nc, x_chunks, aux, out, G)
    nc.finalize()
    return nc


def _get_nc(G=None):
    if G is None:
        G = _CACHE.get("G", G_FULL)
    key = ("nc", G)
    if key not in _CACHE:
        _CACHE[key] = _build(G)
    return _CACHE[key]


def _get_sharded_fn(G):
    """bass_jit kernel shard_mapped over the 8 cores (the proven exec path)."""
    key = ("fn", G)
    if key in _CACHE:
        return _CACHE[key]
    import jax
    from jax.sharding import Mesh, PartitionSpec
    from concourse.bass2jax import bass_jit, bass_shard_map
    from concourse import mybir

    f32 = mybir.dt.float32
    n_chunks = len(_chunk_bounds(G))

    # bass_jit inspects the body's signature, so generate explicit args
    arg_names = [f"x{k}" for k in range(n_chunks)] + ["aux"]
    src = (
        f"def _body(nc, {', '.join(arg_names)}):\n"
        f"    out = nc.dram_tensor('out', [2, 2 * D], f32, kind='ExternalOutput')\n"
        f"    _tile_program(nc, [{', '.join(arg_names[:-1])}], aux, out, G)\n"
        f"    return out\n"
    )
    ns = {"D": D, "f32": f32, "_tile_program": _tile_program, "G": G}
    exec(src, ns)
    body = bass_jit(ns["_body"])

    devices = jax.devices()[:N_CORES]
    mesh = Mesh(np.asarray(devices), ("core",))
    fn = bass_shard_map(
        body,
        mesh=mesh,
        in_specs=tuple(PartitionSpec("core") for _ in range(n_chunks + 1)),
        out_specs=PartitionSpec("core"),
    )
    _CACHE[key] = fn
    return fn


def _make_in_maps(logits, labels, entity_id):
    from concourse import mybir

    BF16 = mybir.dt.np(mybir.dt.bfloat16)

    lg = np.asarray(logits).astype(np.float32, copy=False).reshape(B * S, D)
    labels = np.asarray(labels).reshape(B, S)
    eid = int(np.asarray(entity_id))

    pos_ok = np.arange(S)[None, :] != 0
    ent = ((labels == eid) & pos_ok).reshape(-1)
    nz = (labels != 0).reshape(-1)
    c1 = max(float(ent.sum()), 1.0)
    c2 = max(float(nz.sum()), 1.0)

    # drop tokens that contribute to neither sum
    keep = nz | ent
    idx = np.nonzero(keep)[0]
    K = idx.size
    G = max(-(-K // (N_CORES * P)), 1)
    cap = N_CORES * P * G

    xk = np.ones((cap, D), dtype=BF16)           # pad rows: nonzero norm
    entk = np.zeros(cap, dtype=BF16)
    nzk = np.zeros(cap, dtype=BF16)
    xk[:K] = lg[idx].astype(BF16)
    entk[:K] = ent[idx].astype(BF16)
    nzk[:K] = nz[idx].astype(BF16)

    bounds = _chunk_bounds(G)
    tok_per_core = P * G
    in_maps = []
    for c in range(N_CORES):
        sl = slice(c * tok_per_core, (c + 1) * tok_per_core)
        x = xk[sl].reshape(G, P, D).transpose(1, 0, 2)  # [P, G, D] view
        m = {
            f"x{k}": np.ascontiguousarray(x[:, a:b, :])
            for k, (a, b) in enumerate(bounds)
        }
        m["aux"] = np.ascontiguousarray(
            np.stack([entk[sl], nzk[sl]], axis=-1)
            .reshape(G, P, 2)
            .transpose(1, 0, 2)
        )  # [P, G, 2]
        in_maps.append(m)

    _CACHE["G"] = G
    return in_maps, c1, c2


def _combine(partials, c1, c2):
    """partials: list of [2, D] float arrays (one per core)."""
    acc = np.zeros((2, D), dtype=np.float64)
    for p in partials:
        p = np.asarray(p, dtype=np.float64)
        if p.shape[-1] == 2 * D:
            p = p[:, 0:D] + p[:, D : 2 * D]
        acc += p
    v1, v2 = acc[0], acc[1]
    proto = v1 / c1
    pn = float(np.sqrt((proto * proto).sum()))
    if pn < 1e-30:
        return np.float32(0.0)
    loss = float(v2 @ proto) / (pn * c2)
    return np.float32(loss)


def _run_hw(in_maps):
    """Run the 8-core shard_map; returns list of [2, D] partials."""
    G = _CACHE.get("G", G_FULL)
    fn = _get_sharded_fn(G)
    names = [f"x{k}" for k in range(len(_chunk_bounds(G)))] + ["aux"]
    args = [
        np.concatenate([m[name] for m in in_maps], axis=0) for name in names
    ]
    out = np.asarray(fn(*args))  # [2 * N_CORES, 2 * D]
    return [out[2 * c : 2 * c + 2] for c in range(N_CORES)]


def kernel(logits, labels, entity_id):
    in_maps, c1, c2 = _make_in_maps(logits, labels, entity_id)
    partials = _run_hw(in_maps)
    return _combine(partials, c1, c2)
